# revision 1
# baseline (speedup 1.0000x reference)
"""DMPNN (atom-message) Bass kernel for 8 Trainium2 NeuronCores.

Strategy: dst-block node sharding across 8 cores. Per depth step, each core
gathers message rows for its incoming edges via indirect DMA, reduces them
per 128-node block with one-hot selection matmuls accumulating in PSUM,
applies the W_h update, and AllGathers the new messages (4-way split,
ping-pong buffers so collectives overlap the gather stream).

Messages are stored fp16 with a per-step 1/16 rescaling (the scale factor is
folded back into the W_h / W_o matmuls), keeping values in fp16 range while
all accumulation stays in fp32 PSUM. Node blocks are degree-balanced on the
host so every block has <= 2048 incoming edges (16 gather chunks).
"""

import sys

sys.path.insert(0, '/opt/trn_rl_repo')

import numpy as np

N = 20000
E = 320000
A = 256
B = 128
DH = 512
DEPTH = 6
NC_ = 8          # cores
NB = 20          # node blocks per core
CH = 16          # edge chunks (of 128) per block
BLK = 128
GRP = 5          # blocks per all-gather group
NSPLIT = NB // GRP
NPAD = NC_ * NB * BLK      # 20480
NLOC = NB * BLK            # 2560 nodes per core
EBLK = CH * BLK            # 2048 edge slots per block

_compiled = None


def _partition_nodes(dst):
    """Greedy-balance nodes into NC_*NB bins (<=128 nodes, ~2000 edges each).

    Returns (bin_of_node, slot_of_node).
    """
    import heapq

    nbins = NC_ * NB
    deg = np.bincount(dst, minlength=N)
    order = np.argsort(-deg, kind='stable')
    bin_edges = np.zeros(nbins, dtype=np.int64)
    bin_nodes = np.zeros(nbins, dtype=np.int64)
    bin_of = np.empty(N, dtype=np.int64)
    slot_of = np.empty(N, dtype=np.int64)
    heap = [(0, i) for i in range(nbins)]
    heapq.heapify(heap)
    for node in order:
        while True:
            e, b = heapq.heappop(heap)
            if bin_nodes[b] < BLK:
                break
        bin_of[node] = b
        slot_of[node] = bin_nodes[b]
        bin_nodes[b] += 1
        bin_edges[b] += deg[node]
        if bin_nodes[b] < BLK:
            heapq.heappush(heap, (int(bin_edges[b]), b))
    assert bin_edges.max() <= EBLK, f"block overflow: {bin_edges.max()}"
    return bin_of, slot_of


def _global_newid(core, block, slot):
    # Must match the AllGather output layout: group-major, then rank.
    g = block // GRP
    return g * (NC_ * GRP * BLK) + core * (GRP * BLK) + (block % GRP) * BLK + slot


def _build_program():
    from concourse import bass, bacc, tile, mybir

    dt = mybir.dt
    nc = bacc.Bacc("TRN2", target_bir_lowering=False, debug=False,
                   num_devices=NC_)

    atomT0 = nc.dram_tensor("atomT0", [BLK, NLOC], dt.float16, kind="ExternalInput")
    atomT1 = nc.dram_tensor("atomT1", [BLK, NLOC], dt.float16, kind="ExternalInput")
    wi = nc.dram_tensor("wi", [A, DH], dt.float16, kind="ExternalInput")
    wh = nc.dram_tensor("wh", [B + DH, DH], dt.float16, kind="ExternalInput")
    wo = nc.dram_tensor("wo", [A + DH, DH], dt.float16, kind="ExternalInput")
    bo = nc.dram_tensor("bo", [BLK, DH], dt.float32, kind="ExternalInput")
    esrc = nc.dram_tensor("esrc", [BLK, NB * CH], dt.int32, kind="ExternalInput")
    eldst = nc.dram_tensor("eldst", [BLK, NB * CH], dt.float16, kind="ExternalInput")
    bondp = nc.dram_tensor("bondp", [NB * BLK, CH * B], dt.float16, kind="ExternalInput")
    out = nc.dram_tensor("out", [NLOC, DH], dt.float32, kind="ExternalOutput")

    msg = [
        nc.dram_tensor("msgA", [NPAD, DH], dt.float16, addr_space="Shared"),
        nc.dram_tensor("msgB", [NPAD, DH], dt.float16, addr_space="Shared"),
    ]
    ag_in = nc.dram_tensor("ag_in", [NLOC, DH], dt.float16)

    iota_np = np.broadcast_to(np.arange(BLK, dtype=np.float16), (BLK, BLK)).copy()
    iota_t = nc.inline_tensor(iota_np, "iota")
    ident_t = nc.inline_tensor(np.eye(BLK, dtype=np.float16), "ident")

    groups = [list(range(NC_))]

    with tile.TileContext(nc) as tc:
        with (
            tc.tile_pool(name="const", bufs=1) as cp,
            tc.tile_pool(name="work", bufs=2) as wp,
            tc.tile_pool(name="psum", bufs=2, space="PSUM") as pp,
        ):
            # ---- resident constants ----
            iota_sb = cp.tile([BLK, BLK], dt.float16, tag="iota")
            nc.sync.dma_start(out=iota_sb[:], in_=iota_t[:])
            ident_sb = cp.tile([BLK, BLK], dt.float16, tag="ident")
            nc.sync.dma_start(out=ident_sb[:], in_=ident_t[:])
            wi_sb = cp.tile([BLK, 2 * DH], dt.float16, tag="wi")
            for k in range(2):
                nc.sync.dma_start(out=wi_sb[:, k * DH:(k + 1) * DH],
                                  in_=wi[k * BLK:(k + 1) * BLK, :])
            wh_sb = cp.tile([BLK, 5 * DH], dt.float16, tag="wh")
            for k in range(5):
                nc.sync.dma_start(out=wh_sb[:, k * DH:(k + 1) * DH],
                                  in_=wh[k * BLK:(k + 1) * BLK, :])
            wo_sb = cp.tile([BLK, 6 * DH], dt.float16, tag="wo")
            for k in range(6):
                nc.sync.dma_start(out=wo_sb[:, k * DH:(k + 1) * DH],
                                  in_=wo[k * BLK:(k + 1) * BLK, :])
            bo_sb = cp.tile([BLK, DH], dt.float32, tag="bo")
            nc.sync.dma_start(out=bo_sb[:], in_=bo[:])
            at0_sb = cp.tile([BLK, NLOC], dt.float16, tag="at0")
            nc.sync.dma_start(out=at0_sb[:], in_=atomT0[:])
            at1_sb = cp.tile([BLK, NLOC], dt.float16, tag="at1")
            nc.sync.dma_start(out=at1_sb[:], in_=atomT1[:])
            esrc_sb = cp.tile([BLK, NB * CH], dt.int32, tag="esrc")
            nc.sync.dma_start(out=esrc_sb[:], in_=esrc[:])
            eldst_sb = cp.tile([BLK, NB * CH], dt.float16, tag="eldst")
            nc.sync.dma_start(out=eldst_sb[:], in_=eldst[:])
            inp_sb = cp.tile([BLK, NB * DH], dt.float32, tag="inp")
            sumeT_sb = cp.tile([BLK, NB * BLK], dt.float16, tag="sumeT")

            def onehot_for(col):
                oh = wp.tile([BLK, BLK], dt.float16, tag="oh")
                nc.vector.tensor_tensor(
                    out=oh[:],
                    in0=eldst_sb[:, col:col + 1].to_broadcast([BLK, BLK]),
                    in1=iota_sb[:],
                    op=mybir.AluOpType.is_equal,
                )
                return oh

            def ag(g, step_dst):
                nc.gpsimd.collective_compute(
                    "AllGather",
                    mybir.AluOpType.bypass,
                    replica_groups=groups,
                    ins=[ag_in[g * GRP * BLK:(g + 1) * GRP * BLK, :]],
                    outs=[step_dst[g * NC_ * GRP * BLK:(g + 1) * NC_ * GRP * BLK, :]],
                )

            # ---- step 1: inp = atom @ W_i ; msg1 = relu(inp) ----
            for b in range(NB):
                ps = pp.tile([BLK, DH], dt.float32, tag="z")
                for k in range(2):
                    nc.tensor.matmul(
                        out=ps[:],
                        lhsT=(at0_sb if k == 0 else at1_sb)[:, b * BLK:(b + 1) * BLK],
                        rhs=wi_sb[:, k * DH:(k + 1) * DH],
                        start=(k == 0), stop=(k == 1),
                    )
                nc.vector.tensor_copy(out=inp_sb[:, b * DH:(b + 1) * DH], in_=ps[:])
                m = wp.tile([BLK, DH], dt.float16, tag="msg")
                nc.vector.tensor_scalar_max(out=m[:], in0=ps[:], scalar1=0.0)
                nc.sync.dma_start(out=ag_in[b * BLK:(b + 1) * BLK, :], in_=m[:])
                if b % GRP == GRP - 1:
                    ag(b // GRP, msg[0])


            # ---- sum_e (constant across steps), stored transposed ----
            for b in range(NB):
                ps = pp.tile([BLK, BLK], dt.float32, tag="se")
                bt = wp.tile([BLK, CH * B], dt.float16, tag="bond", bufs=2)
                nc.sync.dma_start(out=bt[:], in_=bondp[b * BLK:(b + 1) * BLK, :])
                for c in range(CH):
                    oh = onehot_for(b * CH + c)
                    nc.tensor.matmul(out=ps[:], lhsT=oh[:],
                                     rhs=bt[:, c * B:(c + 1) * B],
                                     start=(c == 0), stop=(c == CH - 1))
                se = wp.tile([BLK, BLK], dt.float16, tag="se")
                nc.vector.tensor_copy(out=se[:], in_=ps[:])
                ps2 = pp.tile([BLK, BLK], dt.float16, tag="tp")
                nc.tensor.transpose(out=ps2[:], in_=se[:], identity=ident_sb[:])
                nc.vector.tensor_copy(out=sumeT_sb[:, b * BLK:(b + 1) * BLK], in_=ps2[:])

            # ---- steps 2..DEPTH+1 (5 x W_h update, then W_o readout) ----
            for s in range(2, DEPTH + 2):
                src_buf = msg[(s - 2) % 2]
                dst_buf = msg[(s - 1) % 2]
                final = (s == DEPTH + 1)
                whf = 16.0 ** (s - 2)
                if final:
                    ws = wp.tile([BLK, 4 * DH], dt.float16, tag="whs", bufs=2)
                    for j in range(4):
                        nc.vector.tensor_scalar_mul(
                            out=ws[:, j * DH:(j + 1) * DH],
                            in0=wo_sb[:, (2 + j) * DH:(3 + j) * DH],
                            scalar1=whf / 16.0)
                elif s > 2:
                    ws = wp.tile([BLK, 4 * DH], dt.float16, tag="whs", bufs=2)
                    for j in range(4):
                        nc.vector.tensor_scalar_mul(
                            out=ws[:, j * DH:(j + 1) * DH],
                            in0=wh_sb[:, j * DH:(j + 1) * DH], scalar1=whf)
                else:
                    ws = None
                for b in range(NB):
                    ps = pp.tile([BLK, DH], dt.float32, tag="sh")
                    for c in range(CH):
                        col = b * CH + c
                        G = wp.tile([BLK, DH], dt.float16, tag="g", bufs=16)
                        nc.gpsimd.indirect_dma_start(
                            out=G[:], out_offset=None, in_=src_buf[:],
                            in_offset=bass.IndirectOffsetOnAxis(
                                ap=esrc_sb[:, col:col + 1], axis=0),
                        )
                        oh = onehot_for(col)
                        nc.tensor.matmul(out=ps[:], lhsT=oh[:], rhs=G[:],
                                         start=(c == 0), stop=(c == CH - 1))
                    sh = wp.tile([BLK, DH], dt.float16, tag="shs")
                    nc.vector.tensor_copy(out=sh[:], in_=ps[:])
                    pt = pp.tile([BLK, DH], dt.float16, tag="tp")
                    for j in range(4):
                        nc.tensor.transpose(out=pt[:, j * BLK:(j + 1) * BLK],
                                            in_=sh[:, j * BLK:(j + 1) * BLK],
                                            identity=ident_sb[:])
                    shT = wp.tile([BLK, DH], dt.float16, tag="shT")
                    if final:
                        nc.vector.tensor_scalar_mul(out=shT[:], in0=pt[:], scalar1=16.0)
                    else:
                        nc.vector.tensor_copy(out=shT[:], in_=pt[:])

                    pz = pp.tile([BLK, DH], dt.float32, tag="z")
                    if final:
                        for k in range(2):
                            nc.tensor.matmul(
                                out=pz[:],
                                lhsT=(at0_sb if k == 0 else at1_sb)[:, b * BLK:(b + 1) * BLK],
                                rhs=wo_sb[:, k * DH:(k + 1) * DH],
                                start=(k == 0), stop=False)
                        for j in range(4):
                            nc.tensor.matmul(
                                out=pz[:], lhsT=shT[:, j * BLK:(j + 1) * BLK],
                                rhs=ws[:, j * DH:(j + 1) * DH],
                                start=False, stop=(j == 3))
                        zz = wp.tile([BLK, DH], dt.float32, tag="zz")
                        nc.vector.tensor_tensor(out=zz[:], in0=pz[:], in1=bo_sb[:],
                                                op=mybir.AluOpType.add)
                        o = wp.tile([BLK, DH], dt.float32, tag="out")
                        nc.vector.tensor_scalar_max(out=o[:], in0=zz[:], scalar1=0.0)
                        nc.sync.dma_start(out=out[b * BLK:(b + 1) * BLK, :], in_=o[:])
                    else:
                        for j in range(4):
                            nc.tensor.matmul(
                                out=pz[:], lhsT=shT[:, j * BLK:(j + 1) * BLK],
                                rhs=(ws if ws is not None else wh_sb)[:, j * DH:(j + 1) * DH],
                                start=(j == 0), stop=False)
                        nc.tensor.matmul(
                            out=pz[:], lhsT=sumeT_sb[:, b * BLK:(b + 1) * BLK],
                            rhs=wh_sb[:, 4 * DH:5 * DH],
                            start=False, stop=True)
                        zz = wp.tile([BLK, DH], dt.float32, tag="zz")
                        nc.vector.tensor_tensor(out=zz[:], in0=pz[:],
                                                in1=inp_sb[:, b * DH:(b + 1) * DH],
                                                op=mybir.AluOpType.add)
                        m = wp.tile([BLK, DH], dt.float16, tag="msg")
                        nc.vector.tensor_scalar(
                            out=m[:], in0=zz[:], scalar1=0.0,
                            scalar2=16.0 ** (1 - s), op0=mybir.AluOpType.max,
                            op1=mybir.AluOpType.mult)
                        nc.sync.dma_start(out=ag_in[b * BLK:(b + 1) * BLK, :], in_=m[:])
                        if b % GRP == GRP - 1:
                            ag(b // GRP, dst_buf)

    nc.compile()
    return nc


def _prep_inputs(atom, bond, src, dst, W_i, W_h, W_o, b_o):
    bin_of, slot_of = _partition_nodes(dst)
    core_of_bin = np.arange(NC_ * NB) // NB
    block_of_bin = np.arange(NC_ * NB) % NB

    # global new id per node (matches AG layout)
    nbin = bin_of
    gid = _global_newid(core_of_bin[nbin], block_of_bin[nbin], slot_of)

    # per-edge placement
    ebin = bin_of[dst]
    eldst_v = slot_of[dst]
    esrc_v = gid[src]

    order = np.lexsort((esrc_v, ebin))
    ebin_s = ebin[order]
    esrc_s = esrc_v[order]
    eldst_s = eldst_v[order]
    eorig_s = order

    nbins = NC_ * NB
    counts = np.bincount(ebin_s, minlength=nbins)
    offs = np.concatenate([[0], np.cumsum(counts)])

    esrc_pad = np.zeros((nbins, EBLK), dtype=np.int32)
    eldst_pad = np.full((nbins, EBLK), -1.0, dtype=np.float32)
    eorig_pad = np.full((nbins, EBLK), -1, dtype=np.int64)
    for k in range(nbins):
        n = counts[k]
        esrc_pad[k, :n] = esrc_s[offs[k]:offs[k] + n]
        eldst_pad[k, :n] = eldst_s[offs[k]:offs[k] + n]
        eorig_pad[k, :n] = eorig_s[offs[k]:offs[k] + n]

    # atom in per-core local order
    atom_loc = np.zeros((NC_, NLOC, A), dtype=np.float32)
    loc = block_of_bin[nbin] * BLK + slot_of
    atom_loc[core_of_bin[nbin], loc] = atom

    bond0 = np.concatenate([bond.astype(np.float32),
                            np.zeros((1, B), dtype=np.float32)])

    in_maps = []
    for c in range(NC_):
        sl = slice(c * NB, (c + 1) * NB)
        es = esrc_pad[sl].reshape(NB, CH, BLK)        # [b, chunk, slot]
        ed = eldst_pad[sl].reshape(NB, CH, BLK)
        eo = eorig_pad[sl].reshape(NB * CH * BLK)
        # device layout [slot(partition), b*CH + chunk]
        es_dev = np.ascontiguousarray(es.transpose(2, 0, 1).reshape(BLK, NB * CH))
        ed_dev = np.ascontiguousarray(ed.transpose(2, 0, 1).reshape(BLK, NB * CH))
        bp = bond0[eo].reshape(NB, CH, BLK, B).transpose(0, 2, 1, 3)\
            .reshape(NB * BLK, CH * B)                # [NB*BLK, CH*B]
        at = atom_loc[c].T                            # [A, NLOC]
        in_maps.append({
            "atomT0": np.ascontiguousarray(at[:BLK]).astype(np.float16),
            "atomT1": np.ascontiguousarray(at[BLK:]).astype(np.float16),
            "wi": np.ascontiguousarray(W_i.astype(np.float16)),
            "wh": np.ascontiguousarray(W_h.astype(np.float16)),
            "wo": np.ascontiguousarray(W_o.astype(np.float16)),
            "bo": np.ascontiguousarray(
                np.broadcast_to(b_o.astype(np.float32), (BLK, DH))),
            "esrc": es_dev.astype(np.int32),
            "eldst": ed_dev.astype(np.float16),
            "bondp": np.ascontiguousarray(bp).astype(np.float16),
        })

    # map device outputs back: out rows are local order per core
    core_idx = core_of_bin[nbin]
    return in_maps, core_idx, loc


def kernel(**inputs):
    global _compiled
    from concourse.bass_utils import run_bass_kernel_spmd

    atom = np.asarray(inputs["atom"], dtype=np.float32)
    bond = np.asarray(inputs["bond"], dtype=np.float32)
    src = np.asarray(inputs["src"]).astype(np.int64)
    dst = np.asarray(inputs["dst"]).astype(np.int64)

    in_maps, core_idx, loc = _prep_inputs(
        atom, bond, src, dst,
        np.asarray(inputs["W_i"]), np.asarray(inputs["W_h"]),
        np.asarray(inputs["W_o"]), np.asarray(inputs["b_o"]))

    if _compiled is None:
        _compiled = _build_program()

    res = run_bass_kernel_spmd(_compiled, in_maps, list(range(NC_)))
    outs = np.stack([res.results[c]["out"] for c in range(NC_)])  # [NC_, NLOC, DH]
    full = outs[core_idx, loc]                                    # [N, DH]
    return full.astype(np.float32)



# revision 11
# speedup vs baseline: 1.1947x; 1.1947x over previous
"""DMPNN (atom-message) Bass kernel for 8 Trainium2 NeuronCores.

Strategy: dst-block node sharding across 8 cores. Per depth step, each core
gathers message rows for its incoming edges with one dma_gather per
128-node block (2048 indices per call, amortizing the SWDGE
descriptor-generation cost that dominates a per-chunk indirect-DMA
formulation), reduces them per
block with one-hot selection matmuls accumulating in PSUM, applies the W_h
update, and AllGathers the new messages (4-way split, ping-pong buffers so
collectives overlap the gather stream).

Messages are stored fp16 with a per-step 1/16 rescaling (the scale factor is
folded back into the W_h / W_o matmuls), keeping values in fp16 range while
all accumulation stays in fp32 PSUM. The depth-invariant part of the update
(inp + sum_e @ Wh2) is computed once into a resident fp32 "base" tile, so
the per-step work is only gather + scatter-sum + 4 W_h matmuls per block.
Node blocks are degree-balanced on the host so every block has <= 2048
incoming edges (16 chunks of 128).
"""

import sys

sys.path.insert(0, '/opt/trn_rl_repo')

import numpy as np

N = 20000
E = 320000
A = 256
B = 128
DH = 512
DEPTH = 6
NC_ = 8          # cores
NB = 20          # node blocks per core
CH = 16          # edge chunks (of 128) per block
BLK = 128
GRP = 5          # blocks per all-gather group
NSPLIT = NB // GRP
NPAD = NC_ * NB * BLK      # 20480
NLOC = NB * BLK            # 2560 nodes per core
EBLK = CH * BLK            # 2048 edge slots per block

_compiled = None


def _partition_nodes(dst):
    """Greedy-balance nodes into NC_*NB bins (<=128 nodes, ~2000 edges each).

    Returns (bin_of_node, slot_of_node).
    """
    import heapq

    nbins = NC_ * NB
    deg = np.bincount(dst, minlength=N)
    order = np.argsort(-deg, kind='stable')
    bin_edges = np.zeros(nbins, dtype=np.int64)
    bin_nodes = np.zeros(nbins, dtype=np.int64)
    bin_of = np.empty(N, dtype=np.int64)
    slot_of = np.empty(N, dtype=np.int64)
    heap = [(0, i) for i in range(nbins)]
    heapq.heapify(heap)
    for node in order:
        while True:
            e, b = heapq.heappop(heap)
            if bin_nodes[b] < BLK:
                break
        bin_of[node] = b
        slot_of[node] = bin_nodes[b]
        bin_nodes[b] += 1
        bin_edges[b] += deg[node]
        if bin_nodes[b] < BLK:
            heapq.heappush(heap, (int(bin_edges[b]), b))
    assert bin_edges.max() <= EBLK, f"block overflow: {bin_edges.max()}"
    return bin_of, slot_of


def _global_newid(core, block, slot):
    # Must match the AllGather output layout: group-major, then rank.
    g = block // GRP
    return g * (NC_ * GRP * BLK) + core * (GRP * BLK) + (block % GRP) * BLK + slot


def _build_program():
    from concourse import bass, bacc, tile, mybir

    dt = mybir.dt
    nc = bacc.Bacc("TRN2", target_bir_lowering=False, debug=False,
                   num_devices=NC_)

    atomT0 = nc.dram_tensor("atomT0", [BLK, NLOC], dt.float16, kind="ExternalInput")
    atomT1 = nc.dram_tensor("atomT1", [BLK, NLOC], dt.float16, kind="ExternalInput")
    wi = nc.dram_tensor("wi", [A, DH], dt.float16, kind="ExternalInput")
    wh = nc.dram_tensor("wh", [B + DH, DH], dt.float16, kind="ExternalInput")
    wo = nc.dram_tensor("wo", [A + DH, DH], dt.float16, kind="ExternalInput")
    bo = nc.dram_tensor("bo", [BLK, DH], dt.float32, kind="ExternalInput")
    gidx = nc.dram_tensor("gidx", [BLK, NB * BLK], dt.int16, kind="ExternalInput")
    eldst = nc.dram_tensor("eldst", [BLK, NB * CH], dt.float16, kind="ExternalInput")
    bondp = nc.dram_tensor("bondp", [NB * BLK, CH * B], dt.float16, kind="ExternalInput")
    out = nc.dram_tensor("out", [NLOC, DH], dt.float32, kind="ExternalOutput")

    msg = [
        nc.dram_tensor("msgA", [NPAD, DH], dt.float16, addr_space="Shared"),
        nc.dram_tensor("msgB", [NPAD, DH], dt.float16, addr_space="Shared"),
    ]
    ag_in = nc.dram_tensor("ag_in", [NLOC, DH], dt.float16)

    iota_np = np.broadcast_to(np.arange(BLK, dtype=np.float16), (BLK, BLK)).copy()
    iota_t = nc.inline_tensor(iota_np, "iota")
    ident_t = nc.inline_tensor(np.eye(BLK, dtype=np.float16), "ident")

    groups = [list(range(NC_))]

    with tile.TileContext(nc) as tc:
        with (
            tc.tile_pool(name="const", bufs=1) as cp,
            tc.tile_pool(name="work", bufs=2) as wp,
            tc.tile_pool(name="gpool", bufs=4) as gp,
            tc.tile_pool(name="psum", bufs=2, space="PSUM") as pp,
        ):
            # ---- resident constants ----
            iota_sb = cp.tile([BLK, BLK], dt.float16, tag="iota")
            nc.sync.dma_start(out=iota_sb[:], in_=iota_t[:])
            ident_sb = cp.tile([BLK, BLK], dt.float16, tag="ident")
            nc.sync.dma_start(out=ident_sb[:], in_=ident_t[:])
            wi_sb = cp.tile([BLK, 2 * DH], dt.float16, tag="wi")
            for k in range(2):
                nc.sync.dma_start(out=wi_sb[:, k * DH:(k + 1) * DH],
                                  in_=wi[k * BLK:(k + 1) * BLK, :])
            wh_sb = cp.tile([BLK, 5 * DH], dt.float16, tag="wh")
            for k in range(5):
                nc.sync.dma_start(out=wh_sb[:, k * DH:(k + 1) * DH],
                                  in_=wh[k * BLK:(k + 1) * BLK, :])
            wo_sb = cp.tile([BLK, 6 * DH], dt.float16, tag="wo")
            for k in range(6):
                nc.sync.dma_start(out=wo_sb[:, k * DH:(k + 1) * DH],
                                  in_=wo[k * BLK:(k + 1) * BLK, :])
            bo_sb = cp.tile([BLK, DH], dt.float32, tag="bo")
            nc.sync.dma_start(out=bo_sb[:], in_=bo[:])
            at0_sb = cp.tile([BLK, NLOC], dt.float16, tag="at0")
            nc.sync.dma_start(out=at0_sb[:], in_=atomT0[:])
            at1_sb = cp.tile([BLK, NLOC], dt.float16, tag="at1")
            nc.sync.dma_start(out=at1_sb[:], in_=atomT1[:])
            gidx_sb = cp.tile([BLK, NB * BLK], dt.int16, tag="gidx")
            nc.sync.dma_start(out=gidx_sb[:], in_=gidx[:])
            eldst_sb = cp.tile([BLK, NB * CH], dt.float16, tag="eldst")
            nc.sync.dma_start(out=eldst_sb[:], in_=eldst[:])
            base_sb = cp.tile([BLK, NB * DH], dt.float32, tag="base")

            def onehot_for(col):
                oh = wp.tile([BLK, BLK], dt.float16, tag="oh")
                nc.vector.tensor_tensor(
                    out=oh[:],
                    in0=eldst_sb[:, col:col + 1].to_broadcast([BLK, BLK]),
                    in1=iota_sb[:],
                    op=mybir.AluOpType.is_equal,
                )
                return oh

            def ag(g, step_dst):
                nc.gpsimd.collective_compute(
                    "AllGather",
                    mybir.AluOpType.bypass,
                    replica_groups=groups,
                    ins=[ag_in[g * GRP * BLK:(g + 1) * GRP * BLK, :]],
                    outs=[step_dst[g * NC_ * GRP * BLK:(g + 1) * NC_ * GRP * BLK, :]],
                )

            # ---- step 1: inp = atom @ W_i ; msg1 = relu(inp); base = inp ----
            for b in range(NB):
                ps = pp.tile([BLK, DH], dt.float32, tag="z")
                for k in range(2):
                    nc.tensor.matmul(
                        out=ps[:],
                        lhsT=(at0_sb if k == 0 else at1_sb)[:, b * BLK:(b + 1) * BLK],
                        rhs=wi_sb[:, k * DH:(k + 1) * DH],
                        start=(k == 0), stop=(k == 1),
                    )
                nc.vector.tensor_copy(out=base_sb[:, b * DH:(b + 1) * DH], in_=ps[:])
                m = wp.tile([BLK, DH], dt.float16, tag="msg")
                nc.vector.tensor_scalar_max(out=m[:], in0=ps[:], scalar1=0.0)
                nc.sync.dma_start(out=ag_in[b * BLK:(b + 1) * BLK, :], in_=m[:])
                if b % GRP == GRP - 1:
                    ag(b // GRP, msg[0])

            # ---- base += sum_e @ Wh2 (constant across steps) ----
            for b in range(NB):
                ps = pp.tile([BLK, BLK], dt.float32, tag="se")
                bt = wp.tile([BLK, CH * B], dt.float16, tag="bond", bufs=2)
                nc.sync.dma_start(out=bt[:], in_=bondp[b * BLK:(b + 1) * BLK, :])
                for c in range(CH):
                    oh = onehot_for(b * CH + c)
                    nc.tensor.matmul(out=ps[:], lhsT=oh[:],
                                     rhs=bt[:, c * B:(c + 1) * B],
                                     start=(c == 0), stop=(c == CH - 1))
                se = wp.tile([BLK, BLK], dt.float16, tag="se")
                nc.vector.tensor_copy(out=se[:], in_=ps[:])
                ps2 = pp.tile([BLK, BLK], dt.float16, tag="tp")
                nc.tensor.transpose(out=ps2[:], in_=se[:], identity=ident_sb[:])
                seT = wp.tile([BLK, BLK], dt.float16, tag="seT")
                nc.vector.tensor_copy(out=seT[:], in_=ps2[:])
                pze = pp.tile([BLK, DH], dt.float32, tag="z")
                nc.tensor.matmul(out=pze[:], lhsT=seT[:],
                                 rhs=wh_sb[:, 4 * DH:5 * DH],
                                 start=True, stop=True)
                nc.vector.tensor_tensor(
                    out=base_sb[:, b * DH:(b + 1) * DH],
                    in0=base_sb[:, b * DH:(b + 1) * DH],
                    in1=pze[:], op=mybir.AluOpType.add)

            # ---- steps 2..DEPTH+1 (5 x W_h update, then W_o readout) ----
            for s in range(2, DEPTH + 2):
                src_buf = msg[(s - 2) % 2]
                dst_buf = msg[(s - 1) % 2]
                final = (s == DEPTH + 1)
                whf = 16.0 ** (s - 2)
                if final:
                    ws = wp.tile([BLK, 4 * DH], dt.float16, tag="whs", bufs=2)
                    for j in range(4):
                        nc.vector.tensor_scalar_mul(
                            out=ws[:, j * DH:(j + 1) * DH],
                            in0=wo_sb[:, (2 + j) * DH:(3 + j) * DH],
                            scalar1=whf / 16.0)
                elif s > 2:
                    ws = wp.tile([BLK, 4 * DH], dt.float16, tag="whs", bufs=2)
                    for j in range(4):
                        nc.vector.tensor_scalar_mul(
                            out=ws[:, j * DH:(j + 1) * DH],
                            in0=wh_sb[:, j * DH:(j + 1) * DH], scalar1=whf)
                else:
                    ws = None
                for b in range(NB):
                    G = gp.tile([BLK, CH, DH], dt.float16, tag="g")
                    nc.gpsimd.dma_gather(
                        G[:], src_buf[:],
                        gidx_sb[:, b * BLK:(b + 1) * BLK],
                        EBLK, EBLK, DH, single_packet=False,
                    )
                    ps = pp.tile([BLK, DH], dt.float32, tag="sh")
                    for c in range(CH):
                        oh = onehot_for(b * CH + c)
                        nc.tensor.matmul(out=ps[:], lhsT=oh[:], rhs=G[:, c, :],
                                         start=(c == 0), stop=(c == CH - 1))
                    sh = wp.tile([BLK, DH], dt.float16, tag="shs")
                    nc.vector.tensor_copy(out=sh[:], in_=ps[:])
                    pt = pp.tile([BLK, DH], dt.float16, tag="tp")
                    for j in range(4):
                        nc.tensor.transpose(out=pt[:, j * BLK:(j + 1) * BLK],
                                            in_=sh[:, j * BLK:(j + 1) * BLK],
                                            identity=ident_sb[:])
                    shT = wp.tile([BLK, DH], dt.float16, tag="shT")
                    if final:
                        nc.vector.tensor_scalar_mul(out=shT[:], in0=pt[:], scalar1=16.0)
                    else:
                        nc.vector.tensor_copy(out=shT[:], in_=pt[:])

                    pz = pp.tile([BLK, DH], dt.float32, tag="z")
                    if final:
                        for k in range(2):
                            nc.tensor.matmul(
                                out=pz[:],
                                lhsT=(at0_sb if k == 0 else at1_sb)[:, b * BLK:(b + 1) * BLK],
                                rhs=wo_sb[:, k * DH:(k + 1) * DH],
                                start=(k == 0), stop=False)
                        for j in range(4):
                            nc.tensor.matmul(
                                out=pz[:], lhsT=shT[:, j * BLK:(j + 1) * BLK],
                                rhs=ws[:, j * DH:(j + 1) * DH],
                                start=False, stop=(j == 3))
                        zz = wp.tile([BLK, DH], dt.float32, tag="zz")
                        nc.vector.tensor_tensor(out=zz[:], in0=pz[:], in1=bo_sb[:],
                                                op=mybir.AluOpType.add)
                        o = wp.tile([BLK, DH], dt.float32, tag="out")
                        nc.vector.tensor_scalar_max(out=o[:], in0=zz[:], scalar1=0.0)
                        nc.sync.dma_start(out=out[b * BLK:(b + 1) * BLK, :], in_=o[:])
                    else:
                        for j in range(4):
                            nc.tensor.matmul(
                                out=pz[:], lhsT=shT[:, j * BLK:(j + 1) * BLK],
                                rhs=(ws if ws is not None else wh_sb)[:, j * DH:(j + 1) * DH],
                                start=(j == 0), stop=(j == 3))
                        zz = wp.tile([BLK, DH], dt.float32, tag="zz")
                        nc.vector.tensor_tensor(out=zz[:], in0=pz[:],
                                                in1=base_sb[:, b * DH:(b + 1) * DH],
                                                op=mybir.AluOpType.add)
                        m = wp.tile([BLK, DH], dt.float16, tag="msg")
                        nc.vector.tensor_scalar(
                            out=m[:], in0=zz[:], scalar1=0.0,
                            scalar2=16.0 ** (1 - s), op0=mybir.AluOpType.max,
                            op1=mybir.AluOpType.mult)
                        nc.sync.dma_start(out=ag_in[b * BLK:(b + 1) * BLK, :], in_=m[:])
                        if b % GRP == GRP - 1:
                            ag(b // GRP, dst_buf)

    nc.compile()
    return nc


def _prep_inputs(atom, bond, src, dst, W_i, W_h, W_o, b_o):
    bin_of, slot_of = _partition_nodes(dst)
    core_of_bin = np.arange(NC_ * NB) // NB
    block_of_bin = np.arange(NC_ * NB) % NB

    # global new id per node (matches AG layout)
    nbin = bin_of
    gid = _global_newid(core_of_bin[nbin], block_of_bin[nbin], slot_of)

    # per-edge placement
    ebin = bin_of[dst]
    eldst_v = slot_of[dst]
    esrc_v = gid[src]

    order = np.lexsort((esrc_v, ebin))
    ebin_s = ebin[order]
    esrc_s = esrc_v[order]
    eldst_s = eldst_v[order]
    eorig_s = order

    nbins = NC_ * NB
    counts = np.bincount(ebin_s, minlength=nbins)
    offs = np.concatenate([[0], np.cumsum(counts)])

    esrc_pad = np.zeros((nbins, EBLK), dtype=np.int64)
    eldst_pad = np.full((nbins, EBLK), -1.0, dtype=np.float32)
    eorig_pad = np.full((nbins, EBLK), -1, dtype=np.int64)
    for k in range(nbins):
        n = counts[k]
        esrc_pad[k, :n] = esrc_s[offs[k]:offs[k] + n]
        eldst_pad[k, :n] = eldst_s[offs[k]:offs[k] + n]
        eorig_pad[k, :n] = eorig_s[offs[k]:offs[k] + n]

    # atom in per-core local order
    atom_loc = np.zeros((NC_, NLOC, A), dtype=np.float32)
    loc = block_of_bin[nbin] * BLK + slot_of
    atom_loc[core_of_bin[nbin], loc] = atom

    bond0 = np.concatenate([bond.astype(np.float32),
                            np.zeros((1, B), dtype=np.float32)])

    in_maps = []
    for c in range(NC_):
        sl = slice(c * NB, (c + 1) * NB)
        ed = eldst_pad[sl].reshape(NB, CH, BLK)
        eo = eorig_pad[sl].reshape(NB * CH * BLK)
        # gather idx tile: idx i of block b at (partition i % 16, col i // 16)
        gi = esrc_pad[sl].reshape(NB, BLK, 16).transpose(2, 0, 1)  # [16, NB, BLK]
        # replicated across the 8 Q7 cores' 16-partition stripes
        gi_dev = np.tile(gi.reshape(16, NB * BLK), (8, 1)).astype(np.int16)
        # eldst device layout [slot(partition), b*CH + chunk]
        ed_dev = np.ascontiguousarray(ed.transpose(2, 0, 1).reshape(BLK, NB * CH))
        bp = bond0[eo].reshape(NB, CH, BLK, B).transpose(0, 2, 1, 3)\
            .reshape(NB * BLK, CH * B)                # [NB*BLK, CH*B]
        at = atom_loc[c].T                            # [A, NLOC]
        in_maps.append({
            "atomT0": np.ascontiguousarray(at[:BLK]).astype(np.float16),
            "atomT1": np.ascontiguousarray(at[BLK:]).astype(np.float16),
            "wi": np.ascontiguousarray(W_i.astype(np.float16)),
            "wh": np.ascontiguousarray(W_h.astype(np.float16)),
            "wo": np.ascontiguousarray(W_o.astype(np.float16)),
            "bo": np.ascontiguousarray(
                np.broadcast_to(b_o.astype(np.float32), (BLK, DH))),
            "gidx": gi_dev,
            "eldst": ed_dev.astype(np.float16),
            "bondp": np.ascontiguousarray(bp).astype(np.float16),
        })

    # map device outputs back: out rows are local order per core
    core_idx = core_of_bin[nbin]
    return in_maps, core_idx, loc


def kernel(**inputs):
    global _compiled
    from concourse.bass_utils import run_bass_kernel_spmd

    atom = np.asarray(inputs["atom"], dtype=np.float32)
    bond = np.asarray(inputs["bond"], dtype=np.float32)
    src = np.asarray(inputs["src"]).astype(np.int64)
    dst = np.asarray(inputs["dst"]).astype(np.int64)

    in_maps, core_idx, loc = _prep_inputs(
        atom, bond, src, dst,
        np.asarray(inputs["W_i"]), np.asarray(inputs["W_h"]),
        np.asarray(inputs["W_o"]), np.asarray(inputs["b_o"]))

    if _compiled is None:
        _compiled = _build_program()

    res = run_bass_kernel_spmd(_compiled, in_maps, list(range(NC_)))
    outs = np.stack([res.results[c]["out"] for c in range(NC_)])  # [NC_, NLOC, DH]
    full = outs[core_idx, loc]                                    # [N, DH]
    return full.astype(np.float32)


# revision 13
# speedup vs baseline: 1.2070x; 1.0103x over previous
"""DMPNN (atom-message) Bass kernel for 8 Trainium2 NeuronCores.

Strategy: dst-block node sharding across 8 cores. Per depth step, each core
gathers message rows for its incoming edges with one dma_gather per
128-node block (2048 indices per call, amortizing the SWDGE
descriptor-generation cost that dominates a per-chunk indirect-DMA
formulation), reduces them per block with one-hot selection matmuls
accumulating in PSUM, applies the W_h update, and AllGathers the new
messages (uneven 4-way split [7,6,5,2] so the final, critical-path
AllGather of each step is small).

Step 1 (m1 = relu(atom @ W_i)) is computed REPLICATED on every core from a
full copy of the atom matrix, writing msg buffer 0 locally — no collective
needed before the first gather step. The depth-invariant part of the update
(inp + sum_e @ Wh2) is computed once into a resident fp32 "base" tile.

Messages are stored fp16 with a per-step 1/16 rescaling; the scale factor
is folded into host-prescaled copies of W_h / W_o, keeping values in fp16
range while all accumulation stays in fp32 PSUM. Node blocks are
degree-balanced on the host so every block has <= 2048 incoming edges.
"""

import sys

sys.path.insert(0, '/opt/trn_rl_repo')

import numpy as np

N = 20000
E = 320000
A = 256
B = 128
DH = 512
DEPTH = 6
NC_ = 8          # cores
NB = 20          # node blocks per core
CH = 16          # edge chunks (of 128) per block
BLK = 128
GROUP_SIZES = [7, 6, 5, 2]       # blocks per all-gather group
GSTART = [0, 7, 13, 18]
NGRP = len(GROUP_SIZES)
NPAD = NC_ * NB * BLK      # 20480
NLOC = NB * BLK            # 2560 nodes per core
EBLK = CH * BLK            # 2048 edge slots per block
NGB = NC_ * NB             # 160 global blocks

_compiled = None


def _group_of(block):
    for g in range(NGRP - 1, -1, -1):
        if block >= GSTART[g]:
            return g
    return 0


def _partition_nodes(dst):
    """Greedy-balance nodes into NC_*NB bins (<=128 nodes, ~2000 edges each).

    Returns (bin_of_node, slot_of_node).
    """
    import heapq

    nbins = NC_ * NB
    deg = np.bincount(dst, minlength=N)
    order = np.argsort(-deg, kind='stable')
    bin_edges = np.zeros(nbins, dtype=np.int64)
    bin_nodes = np.zeros(nbins, dtype=np.int64)
    bin_of = np.empty(N, dtype=np.int64)
    slot_of = np.empty(N, dtype=np.int64)
    heap = [(0, i) for i in range(nbins)]
    heapq.heapify(heap)
    for node in order:
        while True:
            e, b = heapq.heappop(heap)
            if bin_nodes[b] < BLK:
                break
        bin_of[node] = b
        slot_of[node] = bin_nodes[b]
        bin_nodes[b] += 1
        bin_edges[b] += deg[node]
        if bin_nodes[b] < BLK:
            heapq.heappush(heap, (int(bin_edges[b]), b))
    assert bin_edges.max() <= EBLK, f"block overflow: {bin_edges.max()}"
    return bin_of, slot_of


def _global_newid(core, block, slot):
    # Must match the AllGather output layout: group-major, then rank.
    g = np.vectorize(_group_of)(block) if np.ndim(block) else _group_of(block)
    gs = np.asarray(GROUP_SIZES)[g]
    gstart = np.asarray(GSTART)[g]
    return (NC_ * BLK * gstart + core * gs * BLK
            + (block - gstart) * BLK + slot)


def _build_program():
    from concourse import bass, bacc, tile, mybir

    dt = mybir.dt
    nc = bacc.Bacc("TRN2", target_bir_lowering=False, debug=False,
                   num_devices=NC_)

    atomF0 = nc.dram_tensor("atomF0", [BLK, NPAD], dt.float16, kind="ExternalInput")
    atomF1 = nc.dram_tensor("atomF1", [BLK, NPAD], dt.float16, kind="ExternalInput")
    atomT0 = nc.dram_tensor("atomT0", [BLK, NLOC], dt.float16, kind="ExternalInput")
    atomT1 = nc.dram_tensor("atomT1", [BLK, NLOC], dt.float16, kind="ExternalInput")
    wi = nc.dram_tensor("wi", [A, DH], dt.float16, kind="ExternalInput")
    wh = nc.dram_tensor("wh", [B + DH, DH], dt.float16, kind="ExternalInput")
    whs = nc.dram_tensor("whs", [BLK, 16 * DH], dt.float16, kind="ExternalInput")
    wo = nc.dram_tensor("wo", [A + DH, DH], dt.float16, kind="ExternalInput")
    wos = nc.dram_tensor("wos", [BLK, 4 * DH], dt.float16, kind="ExternalInput")
    bo = nc.dram_tensor("bo", [BLK, DH], dt.float32, kind="ExternalInput")
    gidx = nc.dram_tensor("gidx", [BLK, NB * BLK], dt.int16, kind="ExternalInput")
    eldst = nc.dram_tensor("eldst", [BLK, NB * CH], dt.float16, kind="ExternalInput")
    bondp = nc.dram_tensor("bondp", [NB * BLK, CH * B], dt.float16, kind="ExternalInput")
    out = nc.dram_tensor("out", [NLOC, DH], dt.float32, kind="ExternalOutput")

    msg = [
        nc.dram_tensor("msgA", [NPAD, DH], dt.float16, addr_space="Shared"),
        nc.dram_tensor("msgB", [NPAD, DH], dt.float16, addr_space="Shared"),
    ]
    ag_in = nc.dram_tensor("ag_in", [NLOC, DH], dt.float16)

    iota_np = np.broadcast_to(np.arange(BLK, dtype=np.float16), (BLK, BLK)).copy()
    iota_t = nc.inline_tensor(iota_np, "iota")
    ident_t = nc.inline_tensor(np.eye(BLK, dtype=np.float16), "ident")

    groups = [list(range(NC_))]

    with tile.TileContext(nc) as tc:
        with (
            tc.tile_pool(name="const", bufs=1) as cp,
            tc.tile_pool(name="work", bufs=2) as wp,
            tc.tile_pool(name="gpool", bufs=4) as gp,
            tc.tile_pool(name="psum", bufs=2, space="PSUM") as pp,
        ):
            # ---- resident constants ----
            iota_sb = cp.tile([BLK, BLK], dt.float16, tag="iota")
            nc.sync.dma_start(out=iota_sb[:], in_=iota_t[:])
            ident_sb = cp.tile([BLK, BLK], dt.float16, tag="ident")
            nc.sync.dma_start(out=ident_sb[:], in_=ident_t[:])
            wi_sb = cp.tile([BLK, 2 * DH], dt.float16, tag="wi")
            for k in range(2):
                nc.sync.dma_start(out=wi_sb[:, k * DH:(k + 1) * DH],
                                  in_=wi[k * BLK:(k + 1) * BLK, :])
            wh_sb = cp.tile([BLK, 5 * DH], dt.float16, tag="wh")
            for k in range(5):
                nc.sync.dma_start(out=wh_sb[:, k * DH:(k + 1) * DH],
                                  in_=wh[k * BLK:(k + 1) * BLK, :])
            whs_sb = cp.tile([BLK, 16 * DH], dt.float16, tag="whs")
            nc.sync.dma_start(out=whs_sb[:], in_=whs[:])
            wo_sb = cp.tile([BLK, 2 * DH], dt.float16, tag="wo")
            for k in range(2):
                nc.sync.dma_start(out=wo_sb[:, k * DH:(k + 1) * DH],
                                  in_=wo[k * BLK:(k + 1) * BLK, :])
            wos_sb = cp.tile([BLK, 4 * DH], dt.float16, tag="wos")
            nc.sync.dma_start(out=wos_sb[:], in_=wos[:])
            bo_sb = cp.tile([BLK, DH], dt.float32, tag="bo")
            nc.sync.dma_start(out=bo_sb[:], in_=bo[:])
            at0_sb = cp.tile([BLK, NLOC], dt.float16, tag="at0")
            nc.sync.dma_start(out=at0_sb[:], in_=atomT0[:])
            at1_sb = cp.tile([BLK, NLOC], dt.float16, tag="at1")
            nc.sync.dma_start(out=at1_sb[:], in_=atomT1[:])
            gidx_sb = cp.tile([BLK, NB * BLK], dt.int16, tag="gidx")
            nc.sync.dma_start(out=gidx_sb[:], in_=gidx[:])
            eldst_sb = cp.tile([BLK, NB * CH], dt.float16, tag="eldst")
            nc.sync.dma_start(out=eldst_sb[:], in_=eldst[:])
            base_sb = cp.tile([BLK, NB * DH], dt.float32, tag="base")

            def onehot_block(b):
                """One DVE op building all 16 one-hot tiles for block b."""
                oh = wp.tile([BLK, CH, BLK], dt.float16, tag="oh")
                nc.vector.tensor_tensor(
                    out=oh[:],
                    in0=eldst_sb[:, b * CH:(b + 1) * CH].unsqueeze(2)
                        .to_broadcast([BLK, CH, BLK]),
                    in1=iota_sb[:].unsqueeze(1).to_broadcast([BLK, CH, BLK]),
                    op=mybir.AluOpType.is_equal,
                )
                return oh

            def ag(g, step_dst):
                r0, r1 = GSTART[g] * BLK, (GSTART[g] + GROUP_SIZES[g]) * BLK
                nc.gpsimd.collective_compute(
                    "AllGather",
                    mybir.AluOpType.bypass,
                    replica_groups=groups,
                    ins=[ag_in[r0:r1, :]],
                    outs=[step_dst[NC_ * r0:NC_ * r1, :]],
                )

            # ---- step 1 (replicated): msg0 = relu(atom_all @ W_i) ----
            for q in range(NGB // 4):          # 40 chunks of 4 blocks
                af = wp.tile([BLK, 2, 4 * BLK], dt.float16, tag="af", bufs=3)
                nc.sync.dma_start(out=af[:, 0, :],
                                  in_=atomF0[:, q * 4 * BLK:(q + 1) * 4 * BLK])
                nc.sync.dma_start(out=af[:, 1, :],
                                  in_=atomF1[:, q * 4 * BLK:(q + 1) * 4 * BLK])
                m4 = wp.tile([BLK, 4, DH], dt.float16, tag="m4", bufs=3)
                for j in range(4):
                    ps = pp.tile([BLK, DH], dt.float32, tag="z")
                    for k in range(2):
                        nc.tensor.matmul(
                            out=ps[:],
                            lhsT=af[:, k, j * BLK:(j + 1) * BLK],
                            rhs=wi_sb[:, k * DH:(k + 1) * DH],
                            start=(k == 0), stop=(k == 1),
                        )
                    nc.vector.tensor_scalar_max(out=m4[:, j, :], in0=ps[:],
                                                scalar1=0.0)
                nc.sync.dma_start(
                    out=msg[0][q * 4 * BLK:(q + 1) * 4 * BLK, :]
                        .rearrange("(c p) e -> p c e", p=BLK),
                    in_=m4[:])

            # ---- base = inp (local blocks) ----
            for b in range(NB):
                ps = pp.tile([BLK, DH], dt.float32, tag="z")
                for k in range(2):
                    nc.tensor.matmul(
                        out=ps[:],
                        lhsT=(at0_sb if k == 0 else at1_sb)[:, b * BLK:(b + 1) * BLK],
                        rhs=wi_sb[:, k * DH:(k + 1) * DH],
                        start=(k == 0), stop=(k == 1),
                    )
                nc.vector.tensor_copy(out=base_sb[:, b * DH:(b + 1) * DH], in_=ps[:])

            # ---- base += sum_e @ Wh2 (constant across steps) ----
            for b in range(NB):
                ps = pp.tile([BLK, BLK], dt.float32, tag="se")
                bt = wp.tile([BLK, CH * B], dt.float16, tag="bond", bufs=2)
                nc.sync.dma_start(out=bt[:], in_=bondp[b * BLK:(b + 1) * BLK, :])
                oh = onehot_block(b)
                for c in range(CH):
                    nc.tensor.matmul(out=ps[:], lhsT=oh[:, c, :],
                                     rhs=bt[:, c * B:(c + 1) * B],
                                     start=(c == 0), stop=(c == CH - 1))
                se = wp.tile([BLK, BLK], dt.float16, tag="se")
                nc.vector.tensor_copy(out=se[:], in_=ps[:])
                ps2 = pp.tile([BLK, BLK], dt.float16, tag="tp")
                nc.tensor.transpose(out=ps2[:], in_=se[:], identity=ident_sb[:])
                seT = wp.tile([BLK, BLK], dt.float16, tag="seT")
                nc.vector.tensor_copy(out=seT[:], in_=ps2[:])
                pze = pp.tile([BLK, DH], dt.float32, tag="z")
                nc.tensor.matmul(out=pze[:], lhsT=seT[:],
                                 rhs=wh_sb[:, 4 * DH:5 * DH],
                                 start=True, stop=True)
                nc.vector.tensor_tensor(
                    out=base_sb[:, b * DH:(b + 1) * DH],
                    in0=base_sb[:, b * DH:(b + 1) * DH],
                    in1=pze[:], op=mybir.AluOpType.add)

            # ---- steps 2..DEPTH+1 (5 x W_h update, then W_o readout) ----
            for s in range(2, DEPTH + 2):
                src_buf = msg[(s - 2) % 2]
                dst_buf = msg[(s - 1) % 2]
                final = (s == DEPTH + 1)
                if final:
                    ws = wos_sb
                elif s == 2:
                    ws = wh_sb                      # unscaled, chunks 0..3
                else:
                    ws = whs_sb[:, (s - 3) * 4 * DH:(s - 2) * 4 * DH]
                for b in range(NB):
                    G = gp.tile([BLK, CH, DH], dt.float16, tag="g")
                    nc.gpsimd.dma_gather(
                        G[:], src_buf[:],
                        gidx_sb[:, b * BLK:(b + 1) * BLK],
                        EBLK, EBLK, DH, single_packet=False,
                    )
                    oh = onehot_block(b)
                    ps = pp.tile([BLK, DH], dt.float32, tag="sh")
                    for c in range(CH):
                        nc.tensor.matmul(out=ps[:], lhsT=oh[:, c, :],
                                         rhs=G[:, c, :],
                                         start=(c == 0), stop=(c == CH - 1))
                    sh = wp.tile([BLK, DH], dt.float16, tag="shs")
                    nc.vector.tensor_copy(out=sh[:], in_=ps[:])
                    pt = pp.tile([BLK, DH], dt.float16, tag="tp")
                    for j in range(4):
                        nc.tensor.transpose(out=pt[:, j * BLK:(j + 1) * BLK],
                                            in_=sh[:, j * BLK:(j + 1) * BLK],
                                            identity=ident_sb[:])
                    shT = wp.tile([BLK, DH], dt.float16, tag="shT")
                    if final:
                        nc.vector.tensor_scalar_mul(out=shT[:], in0=pt[:], scalar1=16.0)
                    else:
                        nc.vector.tensor_copy(out=shT[:], in_=pt[:])

                    pz = pp.tile([BLK, DH], dt.float32, tag="z")
                    if final:
                        for k in range(2):
                            nc.tensor.matmul(
                                out=pz[:],
                                lhsT=(at0_sb if k == 0 else at1_sb)[:, b * BLK:(b + 1) * BLK],
                                rhs=wo_sb[:, k * DH:(k + 1) * DH],
                                start=(k == 0), stop=False)
                        for j in range(4):
                            nc.tensor.matmul(
                                out=pz[:], lhsT=shT[:, j * BLK:(j + 1) * BLK],
                                rhs=ws[:, j * DH:(j + 1) * DH],
                                start=False, stop=(j == 3))
                        zz = wp.tile([BLK, DH], dt.float32, tag="zz")
                        nc.vector.tensor_tensor(out=zz[:], in0=pz[:], in1=bo_sb[:],
                                                op=mybir.AluOpType.add)
                        o = wp.tile([BLK, DH], dt.float32, tag="out")
                        nc.vector.tensor_scalar_max(out=o[:], in0=zz[:], scalar1=0.0)
                        nc.sync.dma_start(out=out[b * BLK:(b + 1) * BLK, :], in_=o[:])
                    else:
                        for j in range(4):
                            nc.tensor.matmul(
                                out=pz[:], lhsT=shT[:, j * BLK:(j + 1) * BLK],
                                rhs=ws[:, j * DH:(j + 1) * DH],
                                start=(j == 0), stop=(j == 3))
                        zz = wp.tile([BLK, DH], dt.float32, tag="zz")
                        nc.vector.tensor_tensor(out=zz[:], in0=pz[:],
                                                in1=base_sb[:, b * DH:(b + 1) * DH],
                                                op=mybir.AluOpType.add)
                        m = wp.tile([BLK, DH], dt.float16, tag="msg")
                        nc.vector.tensor_scalar(
                            out=m[:], in0=zz[:], scalar1=0.0,
                            scalar2=16.0 ** (1 - s), op0=mybir.AluOpType.max,
                            op1=mybir.AluOpType.mult)
                        nc.sync.dma_start(out=ag_in[b * BLK:(b + 1) * BLK, :], in_=m[:])
                        g_ = _group_of(b)
                        if b == GSTART[g_] + GROUP_SIZES[g_] - 1:
                            ag(g_, dst_buf)

    nc.compile()
    return nc


def _prep_inputs(atom, bond, src, dst, W_i, W_h, W_o, b_o):
    bin_of, slot_of = _partition_nodes(dst)
    core_of_bin = np.arange(NC_ * NB) // NB
    block_of_bin = np.arange(NC_ * NB) % NB

    # global new id per node (matches AG layout)
    nbin = bin_of
    gid = _global_newid(core_of_bin[nbin], block_of_bin[nbin], slot_of)

    # per-edge placement
    ebin = bin_of[dst]
    eldst_v = slot_of[dst]
    esrc_v = gid[src]

    order = np.lexsort((esrc_v, ebin))
    ebin_s = ebin[order]
    esrc_s = esrc_v[order]
    eldst_s = eldst_v[order]
    eorig_s = order

    nbins = NC_ * NB
    counts = np.bincount(ebin_s, minlength=nbins)
    offs = np.concatenate([[0], np.cumsum(counts)])

    esrc_pad = np.zeros((nbins, EBLK), dtype=np.int64)
    eldst_pad = np.full((nbins, EBLK), -1.0, dtype=np.float32)
    eorig_pad = np.full((nbins, EBLK), -1, dtype=np.int64)
    for k in range(nbins):
        n = counts[k]
        esrc_pad[k, :n] = esrc_s[offs[k]:offs[k] + n]
        eldst_pad[k, :n] = eldst_s[offs[k]:offs[k] + n]
        eorig_pad[k, :n] = eorig_s[offs[k]:offs[k] + n]

    # atom in per-core local order
    atom_loc = np.zeros((NC_, NLOC, A), dtype=np.float32)
    loc = block_of_bin[nbin] * BLK + slot_of
    atom_loc[core_of_bin[nbin], loc] = atom

    # atom in global gid order (for the replicated step-1 pass)
    atom_gid = np.zeros((NPAD, A), dtype=np.float32)
    atom_gid[gid] = atom
    atomF = np.ascontiguousarray(atom_gid.T.astype(np.float16))  # [A, NPAD]

    bond0 = np.concatenate([bond.astype(np.float32),
                            np.zeros((1, B), dtype=np.float32)])

    # host-prescaled step weights
    whs_np = np.zeros((BLK, 16 * DH), dtype=np.float16)
    for si, s in enumerate(range(3, DEPTH + 1)):           # steps 3..6
        f = 16.0 ** (s - 2)
        for j in range(4):
            whs_np[:, (si * 4 + j) * DH:(si * 4 + j + 1) * DH] = (
                W_h[j * BLK:(j + 1) * BLK, :] * f).astype(np.float16)
    wos_np = np.zeros((BLK, 4 * DH), dtype=np.float16)
    f = 16.0 ** 4
    for j in range(4):
        wos_np[:, j * DH:(j + 1) * DH] = \
            (W_o[(2 + j) * BLK:(3 + j) * BLK, :] * f).astype(np.float16)

    in_maps = []
    for c in range(NC_):
        sl = slice(c * NB, (c + 1) * NB)
        ed = eldst_pad[sl].reshape(NB, CH, BLK)
        eo = eorig_pad[sl].reshape(NB * CH * BLK)
        gi = esrc_pad[sl].reshape(NB, BLK, 16).transpose(2, 0, 1)  # [16, NB, BLK]
        # replicated across the 8 Q7 cores' 16-partition stripes
        gi_dev = np.tile(gi.reshape(16, NB * BLK), (8, 1)).astype(np.int16)
        # eldst device layout [slot(partition), b*CH + chunk]
        ed_dev = np.ascontiguousarray(ed.transpose(2, 0, 1).reshape(BLK, NB * CH))
        bp = bond0[eo].reshape(NB, CH, BLK, B).transpose(0, 2, 1, 3)\
            .reshape(NB * BLK, CH * B)                # [NB*BLK, CH*B]
        at = atom_loc[c].T                            # [A, NLOC]
        in_maps.append({
            "atomF0": atomF[:BLK],
            "atomF1": atomF[BLK:],
            "atomT0": np.ascontiguousarray(at[:BLK]).astype(np.float16),
            "atomT1": np.ascontiguousarray(at[BLK:]).astype(np.float16),
            "wi": np.ascontiguousarray(W_i.astype(np.float16)),
            "wh": np.ascontiguousarray(W_h.astype(np.float16)),
            "whs": whs_np,
            "wo": np.ascontiguousarray(W_o.astype(np.float16)),
            "wos": wos_np,
            "bo": np.ascontiguousarray(
                np.broadcast_to(b_o.astype(np.float32), (BLK, DH))),
            "gidx": gi_dev,
            "eldst": ed_dev.astype(np.float16),
            "bondp": np.ascontiguousarray(bp).astype(np.float16),
        })

    # map device outputs back: out rows are local order per core
    core_idx = core_of_bin[nbin]
    return in_maps, core_idx, loc


def kernel(**inputs):
    global _compiled
    from concourse.bass_utils import run_bass_kernel_spmd

    atom = np.asarray(inputs["atom"], dtype=np.float32)
    bond = np.asarray(inputs["bond"], dtype=np.float32)
    src = np.asarray(inputs["src"]).astype(np.int64)
    dst = np.asarray(inputs["dst"]).astype(np.int64)

    in_maps, core_idx, loc = _prep_inputs(
        atom, bond, src, dst,
        np.asarray(inputs["W_i"]), np.asarray(inputs["W_h"]),
        np.asarray(inputs["W_o"]), np.asarray(inputs["b_o"]))

    if _compiled is None:
        _compiled = _build_program()

    res = run_bass_kernel_spmd(_compiled, in_maps, list(range(NC_)))
    outs = np.stack([res.results[c]["out"] for c in range(NC_)])  # [NC_, NLOC, DH]
    full = outs[core_idx, loc]                                    # [N, DH]
    return full.astype(np.float32)


# revision 15
# speedup vs baseline: 1.2111x; 1.0033x over previous
"""DMPNN (atom-message) Bass kernel for 8 Trainium2 NeuronCores.

Strategy: dst-block node sharding across 8 cores. Per depth step, each core
gathers message rows for its incoming edges with one dma_gather per
128-node block (2048 indices per call, amortizing the SWDGE
descriptor-generation cost that dominates a per-chunk indirect-DMA
formulation), reduces them per block with one-hot selection matmuls
accumulating in PSUM, applies the W_h update, and AllGathers the new
messages (uneven 4-way split [7,6,5,2] so the final, critical-path
AllGather of each step is small).

Step 1 (m1 = relu(atom @ W_i)) is computed REPLICATED on every core from a
full copy of the atom matrix, writing msg buffer 0 locally — no collective
needed before the first gather step. The depth-invariant part of the update
(inp + sum_e @ Wh2) is computed once into a resident fp32 "base" tile.

Messages are stored fp16 with a per-step 1/16 rescaling; the scale factor
is folded into host-prescaled copies of W_h / W_o, keeping values in fp16
range while all accumulation stays in fp32 PSUM. Node blocks are
degree-balanced on the host so every block has <= 2048 incoming edges.
"""

import sys

sys.path.insert(0, '/opt/trn_rl_repo')

import numpy as np

N = 20000
E = 320000
A = 256
B = 128
DH = 512
DEPTH = 6
NC_ = 8          # cores
NB = 20          # node blocks per core
CH = 16          # edge chunks (of 128) per block
BLK = 128
GROUP_SIZES = [7, 6, 5, 2]       # blocks per all-gather group
GSTART = [0, 7, 13, 18]
NGRP = len(GROUP_SIZES)
NPAD = NC_ * NB * BLK      # 20480
NLOC = NB * BLK            # 2560 nodes per core
EBLK = CH * BLK            # 2048 edge slots per block
NGB = NC_ * NB             # 160 global blocks

_compiled = None


def _group_of(block):
    for g in range(NGRP - 1, -1, -1):
        if block >= GSTART[g]:
            return g
    return 0


def _partition_nodes(dst):
    """Greedy-balance nodes into NC_*NB bins (<=128 nodes, ~2000 edges each).

    Returns (bin_of_node, slot_of_node).
    """
    import heapq

    nbins = NC_ * NB
    deg = np.bincount(dst, minlength=N)
    order = np.argsort(-deg, kind='stable')
    bin_edges = np.zeros(nbins, dtype=np.int64)
    bin_nodes = np.zeros(nbins, dtype=np.int64)
    bin_of = np.empty(N, dtype=np.int64)
    slot_of = np.empty(N, dtype=np.int64)
    heap = [(0, i) for i in range(nbins)]
    heapq.heapify(heap)
    for node in order:
        while True:
            e, b = heapq.heappop(heap)
            if bin_nodes[b] < BLK:
                break
        bin_of[node] = b
        slot_of[node] = bin_nodes[b]
        bin_nodes[b] += 1
        bin_edges[b] += deg[node]
        if bin_nodes[b] < BLK:
            heapq.heappush(heap, (int(bin_edges[b]), b))
    assert bin_edges.max() <= EBLK, f"block overflow: {bin_edges.max()}"
    return bin_of, slot_of


def _global_newid(core, block, slot):
    # Must match the AllGather output layout: group-major, then rank.
    g = np.vectorize(_group_of)(block) if np.ndim(block) else _group_of(block)
    gs = np.asarray(GROUP_SIZES)[g]
    gstart = np.asarray(GSTART)[g]
    return (NC_ * BLK * gstart + core * gs * BLK
            + (block - gstart) * BLK + slot)


def _build_program():
    from concourse import bass, bacc, tile, mybir

    dt = mybir.dt
    nc = bacc.Bacc("TRN2", target_bir_lowering=False, debug=False,
                   num_devices=NC_)

    atomF0 = nc.dram_tensor("atomF0", [BLK, NPAD], dt.float16, kind="ExternalInput")
    atomF1 = nc.dram_tensor("atomF1", [BLK, NPAD], dt.float16, kind="ExternalInput")
    atomT0 = nc.dram_tensor("atomT0", [BLK, NLOC], dt.float16, kind="ExternalInput")
    atomT1 = nc.dram_tensor("atomT1", [BLK, NLOC], dt.float16, kind="ExternalInput")
    wi = nc.dram_tensor("wi", [A, DH], dt.float16, kind="ExternalInput")
    wh = nc.dram_tensor("wh", [B + DH, DH], dt.float16, kind="ExternalInput")
    whs = nc.dram_tensor("whs", [BLK, 16 * DH], dt.float16, kind="ExternalInput")
    wo = nc.dram_tensor("wo", [A + DH, DH], dt.float16, kind="ExternalInput")
    wos = nc.dram_tensor("wos", [BLK, 4 * DH], dt.float16, kind="ExternalInput")
    bo = nc.dram_tensor("bo", [BLK, DH], dt.float32, kind="ExternalInput")
    gidx = nc.dram_tensor("gidx", [BLK, NB * BLK], dt.int16, kind="ExternalInput")
    eldst = nc.dram_tensor("eldst", [BLK, NB * CH], dt.float16, kind="ExternalInput")
    basep = nc.dram_tensor("basep", [BLK, NB * DH], dt.float32, kind="ExternalInput")
    out = nc.dram_tensor("out", [NLOC, DH], dt.float32, kind="ExternalOutput")

    msg = [
        nc.dram_tensor("msgA", [NPAD, DH], dt.float16, addr_space="Shared"),
        nc.dram_tensor("msgB", [NPAD, DH], dt.float16, addr_space="Shared"),
    ]
    ag_in = nc.dram_tensor("ag_in", [NLOC, DH], dt.float16)

    iota_np = np.broadcast_to(np.arange(BLK, dtype=np.float16), (BLK, BLK)).copy()
    iota_t = nc.inline_tensor(iota_np, "iota")
    ident_t = nc.inline_tensor(np.eye(BLK, dtype=np.float16), "ident")

    groups = [list(range(NC_))]

    with tile.TileContext(nc) as tc:
        with (
            tc.tile_pool(name="const", bufs=1) as cp,
            tc.tile_pool(name="work", bufs=2) as wp,
            tc.tile_pool(name="gpool", bufs=4) as gp,
            tc.tile_pool(name="psum", bufs=2, space="PSUM") as pp,
        ):
            # ---- resident constants ----
            iota_sb = cp.tile([BLK, BLK], dt.float16, tag="iota")
            nc.sync.dma_start(out=iota_sb[:], in_=iota_t[:])
            ident_sb = cp.tile([BLK, BLK], dt.float16, tag="ident")
            nc.sync.dma_start(out=ident_sb[:], in_=ident_t[:])
            wi_sb = cp.tile([BLK, 2 * DH], dt.float16, tag="wi")
            for k in range(2):
                nc.sync.dma_start(out=wi_sb[:, k * DH:(k + 1) * DH],
                                  in_=wi[k * BLK:(k + 1) * BLK, :])
            wh_sb = cp.tile([BLK, 5 * DH], dt.float16, tag="wh")
            for k in range(5):
                nc.sync.dma_start(out=wh_sb[:, k * DH:(k + 1) * DH],
                                  in_=wh[k * BLK:(k + 1) * BLK, :])
            whs_sb = cp.tile([BLK, 16 * DH], dt.float16, tag="whs")
            nc.sync.dma_start(out=whs_sb[:], in_=whs[:])
            wo_sb = cp.tile([BLK, 2 * DH], dt.float16, tag="wo")
            for k in range(2):
                nc.sync.dma_start(out=wo_sb[:, k * DH:(k + 1) * DH],
                                  in_=wo[k * BLK:(k + 1) * BLK, :])
            wos_sb = cp.tile([BLK, 4 * DH], dt.float16, tag="wos")
            nc.sync.dma_start(out=wos_sb[:], in_=wos[:])
            bo_sb = cp.tile([BLK, DH], dt.float32, tag="bo")
            nc.sync.dma_start(out=bo_sb[:], in_=bo[:])
            at0_sb = cp.tile([BLK, NLOC], dt.float16, tag="at0")
            nc.sync.dma_start(out=at0_sb[:], in_=atomT0[:])
            at1_sb = cp.tile([BLK, NLOC], dt.float16, tag="at1")
            nc.sync.dma_start(out=at1_sb[:], in_=atomT1[:])
            gidx_sb = cp.tile([BLK, NB * BLK], dt.int16, tag="gidx")
            nc.sync.dma_start(out=gidx_sb[:], in_=gidx[:])
            eldst_sb = cp.tile([BLK, NB * CH], dt.float16, tag="eldst")
            nc.sync.dma_start(out=eldst_sb[:], in_=eldst[:])
            base_sb = cp.tile([BLK, NB * DH], dt.float32, tag="base")
            nc.sync.dma_start(out=base_sb[:], in_=basep[:])

            def onehot_block(b):
                """One DVE op building all 16 one-hot tiles for block b."""
                oh = wp.tile([BLK, CH, BLK], dt.float16, tag="oh")
                nc.vector.tensor_tensor(
                    out=oh[:],
                    in0=eldst_sb[:, b * CH:(b + 1) * CH].unsqueeze(2)
                        .to_broadcast([BLK, CH, BLK]),
                    in1=iota_sb[:].unsqueeze(1).to_broadcast([BLK, CH, BLK]),
                    op=mybir.AluOpType.is_equal,
                )
                return oh

            def ag(g, step_dst):
                r0, r1 = GSTART[g] * BLK, (GSTART[g] + GROUP_SIZES[g]) * BLK
                nc.gpsimd.collective_compute(
                    "AllGather",
                    mybir.AluOpType.bypass,
                    replica_groups=groups,
                    ins=[ag_in[r0:r1, :]],
                    outs=[step_dst[NC_ * r0:NC_ * r1, :]],
                )

            # ---- step 1 (replicated): msg0 = relu(atom_all @ W_i) ----
            for q in range(NGB // 4):          # 40 chunks of 4 blocks
                af = wp.tile([BLK, 2, 4 * BLK], dt.float16, tag="af", bufs=3)
                nc.sync.dma_start(out=af[:, 0, :],
                                  in_=atomF0[:, q * 4 * BLK:(q + 1) * 4 * BLK])
                nc.sync.dma_start(out=af[:, 1, :],
                                  in_=atomF1[:, q * 4 * BLK:(q + 1) * 4 * BLK])
                m4 = wp.tile([BLK, 4, DH], dt.float16, tag="m4", bufs=3)
                for j in range(4):
                    ps = pp.tile([BLK, DH], dt.float32, tag="z")
                    for k in range(2):
                        nc.tensor.matmul(
                            out=ps[:],
                            lhsT=af[:, k, j * BLK:(j + 1) * BLK],
                            rhs=wi_sb[:, k * DH:(k + 1) * DH],
                            start=(k == 0), stop=(k == 1),
                        )
                    nc.vector.tensor_scalar_max(out=m4[:, j, :], in0=ps[:],
                                                scalar1=0.0)
                nc.sync.dma_start(
                    out=msg[0][q * 4 * BLK:(q + 1) * 4 * BLK, :]
                        .rearrange("(c p) e -> p c e", p=BLK),
                    in_=m4[:])

            # ---- steps 2..DEPTH+1 (5 x W_h update, then W_o readout) ----
            for s in range(2, DEPTH + 2):
                src_buf = msg[(s - 2) % 2]
                dst_buf = msg[(s - 1) % 2]
                final = (s == DEPTH + 1)
                if final:
                    ws = wos_sb
                elif s == 2:
                    ws = wh_sb                      # unscaled, chunks 0..3
                else:
                    ws = whs_sb[:, (s - 3) * 4 * DH:(s - 2) * 4 * DH]
                for b in range(NB):
                    G = gp.tile([BLK, CH, DH], dt.float16, tag="g")
                    nc.gpsimd.dma_gather(
                        G[:], src_buf[:],
                        gidx_sb[:, b * BLK:(b + 1) * BLK],
                        EBLK, EBLK, DH, single_packet=False,
                    )
                    oh = onehot_block(b)
                    ps = pp.tile([BLK, DH], dt.float32, tag="sh")
                    for c in range(CH):
                        nc.tensor.matmul(out=ps[:], lhsT=oh[:, c, :],
                                         rhs=G[:, c, :],
                                         start=(c == 0), stop=(c == CH - 1))
                    sh = wp.tile([BLK, DH], dt.float16, tag="shs")
                    nc.vector.tensor_copy(out=sh[:], in_=ps[:])
                    pt = pp.tile([BLK, DH], dt.float16, tag="tp")
                    for j in range(4):
                        nc.tensor.transpose(out=pt[:, j * BLK:(j + 1) * BLK],
                                            in_=sh[:, j * BLK:(j + 1) * BLK],
                                            identity=ident_sb[:])
                    shT = wp.tile([BLK, DH], dt.float16, tag="shT")
                    if final:
                        nc.vector.tensor_scalar_mul(out=shT[:], in0=pt[:], scalar1=16.0)
                    else:
                        nc.vector.tensor_copy(out=shT[:], in_=pt[:])

                    pz = pp.tile([BLK, DH], dt.float32, tag="z")
                    if final:
                        for k in range(2):
                            nc.tensor.matmul(
                                out=pz[:],
                                lhsT=(at0_sb if k == 0 else at1_sb)[:, b * BLK:(b + 1) * BLK],
                                rhs=wo_sb[:, k * DH:(k + 1) * DH],
                                start=(k == 0), stop=False)
                        for j in range(4):
                            nc.tensor.matmul(
                                out=pz[:], lhsT=shT[:, j * BLK:(j + 1) * BLK],
                                rhs=ws[:, j * DH:(j + 1) * DH],
                                start=False, stop=(j == 3))
                        zz = wp.tile([BLK, DH], dt.float32, tag="zz")
                        nc.vector.tensor_tensor(out=zz[:], in0=pz[:], in1=bo_sb[:],
                                                op=mybir.AluOpType.add)
                        o = wp.tile([BLK, DH], dt.float32, tag="out")
                        nc.vector.tensor_scalar_max(out=o[:], in0=zz[:], scalar1=0.0)
                        nc.sync.dma_start(out=out[b * BLK:(b + 1) * BLK, :], in_=o[:])
                    else:
                        for j in range(4):
                            nc.tensor.matmul(
                                out=pz[:], lhsT=shT[:, j * BLK:(j + 1) * BLK],
                                rhs=ws[:, j * DH:(j + 1) * DH],
                                start=(j == 0), stop=(j == 3))
                        zz = wp.tile([BLK, DH], dt.float32, tag="zz")
                        nc.vector.tensor_tensor(out=zz[:], in0=pz[:],
                                                in1=base_sb[:, b * DH:(b + 1) * DH],
                                                op=mybir.AluOpType.add)
                        m = wp.tile([BLK, DH], dt.float16, tag="msg")
                        nc.vector.tensor_scalar(
                            out=m[:], in0=zz[:], scalar1=0.0,
                            scalar2=16.0 ** (1 - s), op0=mybir.AluOpType.max,
                            op1=mybir.AluOpType.mult)
                        nc.sync.dma_start(out=ag_in[b * BLK:(b + 1) * BLK, :], in_=m[:])
                        for g_ in range(NGRP):
                            end_ = GSTART[g_] + GROUP_SIZES[g_] - 1
                            fire = end_ + 2 if end_ + 2 <= NB - 2 else end_
                            if b == fire:
                                ag(g_, dst_buf)

    nc.compile()
    return nc


def _prep_inputs(atom, bond, src, dst, W_i, W_h, W_o, b_o):
    bin_of, slot_of = _partition_nodes(dst)
    core_of_bin = np.arange(NC_ * NB) // NB
    block_of_bin = np.arange(NC_ * NB) % NB

    # global new id per node (matches AG layout)
    nbin = bin_of
    gid = _global_newid(core_of_bin[nbin], block_of_bin[nbin], slot_of)

    # per-edge placement
    ebin = bin_of[dst]
    eldst_v = slot_of[dst]
    esrc_v = gid[src]

    order = np.lexsort((esrc_v, ebin))
    ebin_s = ebin[order]
    esrc_s = esrc_v[order]
    eldst_s = eldst_v[order]
    eorig_s = order

    nbins = NC_ * NB
    counts = np.bincount(ebin_s, minlength=nbins)
    offs = np.concatenate([[0], np.cumsum(counts)])

    esrc_pad = np.zeros((nbins, EBLK), dtype=np.int64)
    eldst_pad = np.full((nbins, EBLK), -1.0, dtype=np.float32)
    eorig_pad = np.full((nbins, EBLK), -1, dtype=np.int64)
    for k in range(nbins):
        n = counts[k]
        esrc_pad[k, :n] = esrc_s[offs[k]:offs[k] + n]
        eldst_pad[k, :n] = eldst_s[offs[k]:offs[k] + n]
        eorig_pad[k, :n] = eorig_s[offs[k]:offs[k] + n]

    # atom in per-core local order
    atom_loc = np.zeros((NC_, NLOC, A), dtype=np.float32)
    loc = block_of_bin[nbin] * BLK + slot_of
    atom_loc[core_of_bin[nbin], loc] = atom

    # atom in global gid order (for the replicated step-1 pass)
    atom_gid = np.zeros((NPAD, A), dtype=np.float32)
    atom_gid[gid] = atom
    atomF = np.ascontiguousarray(atom_gid.T.astype(np.float16))  # [A, NPAD]

    # host-side depth-invariant base: inp + sum_e @ Wh2, exact fp32
    sum_e = np.zeros((N, B), dtype=np.float32)
    np.add.at(sum_e, dst, bond.astype(np.float32))
    base_full = (atom @ W_i.astype(np.float32)
                 + sum_e @ W_h[DH:, :].astype(np.float32))
    base_loc = np.zeros((NC_, NLOC, DH), dtype=np.float32)
    base_loc[core_of_bin[nbin], loc] = base_full

    # host-prescaled step weights
    whs_np = np.zeros((BLK, 16 * DH), dtype=np.float16)
    for si, s in enumerate(range(3, DEPTH + 1)):           # steps 3..6
        f = 16.0 ** (s - 2)
        for j in range(4):
            whs_np[:, (si * 4 + j) * DH:(si * 4 + j + 1) * DH] = (
                W_h[j * BLK:(j + 1) * BLK, :] * f).astype(np.float16)
    wos_np = np.zeros((BLK, 4 * DH), dtype=np.float16)
    f = 16.0 ** 4
    for j in range(4):
        wos_np[:, j * DH:(j + 1) * DH] = \
            (W_o[(2 + j) * BLK:(3 + j) * BLK, :] * f).astype(np.float16)

    in_maps = []
    for c in range(NC_):
        sl = slice(c * NB, (c + 1) * NB)
        ed = eldst_pad[sl].reshape(NB, CH, BLK)
        gi = esrc_pad[sl].reshape(NB, BLK, 16).transpose(2, 0, 1)  # [16, NB, BLK]
        # replicated across the 8 Q7 cores' 16-partition stripes
        gi_dev = np.tile(gi.reshape(16, NB * BLK), (8, 1)).astype(np.int16)
        # eldst device layout [slot(partition), b*CH + chunk]
        ed_dev = np.ascontiguousarray(ed.transpose(2, 0, 1).reshape(BLK, NB * CH))
        # base in device layout [slot(partition), b*DH + f]
        bl = base_loc[c].reshape(NB, BLK, DH).transpose(1, 0, 2).reshape(BLK, NB * DH)
        at = atom_loc[c].T                            # [A, NLOC]
        in_maps.append({
            "atomF0": atomF[:BLK],
            "atomF1": atomF[BLK:],
            "atomT0": np.ascontiguousarray(at[:BLK]).astype(np.float16),
            "atomT1": np.ascontiguousarray(at[BLK:]).astype(np.float16),
            "wi": np.ascontiguousarray(W_i.astype(np.float16)),
            "wh": np.ascontiguousarray(W_h.astype(np.float16)),
            "whs": whs_np,
            "wo": np.ascontiguousarray(W_o.astype(np.float16)),
            "wos": wos_np,
            "bo": np.ascontiguousarray(
                np.broadcast_to(b_o.astype(np.float32), (BLK, DH))),
            "gidx": gi_dev,
            "eldst": ed_dev.astype(np.float16),
            "basep": np.ascontiguousarray(bl),
        })

    # map device outputs back: out rows are local order per core
    core_idx = core_of_bin[nbin]
    return in_maps, core_idx, loc


def kernel(**inputs):
    global _compiled
    from concourse.bass_utils import run_bass_kernel_spmd

    atom = np.asarray(inputs["atom"], dtype=np.float32)
    bond = np.asarray(inputs["bond"], dtype=np.float32)
    src = np.asarray(inputs["src"]).astype(np.int64)
    dst = np.asarray(inputs["dst"]).astype(np.int64)

    in_maps, core_idx, loc = _prep_inputs(
        atom, bond, src, dst,
        np.asarray(inputs["W_i"]), np.asarray(inputs["W_h"]),
        np.asarray(inputs["W_o"]), np.asarray(inputs["b_o"]))

    if _compiled is None:
        _compiled = _build_program()

    res = run_bass_kernel_spmd(_compiled, in_maps, list(range(NC_)))
    outs = np.stack([res.results[c]["out"] for c in range(NC_)])  # [NC_, NLOC, DH]
    full = outs[core_idx, loc]                                    # [N, DH]
    return full.astype(np.float32)
